# revision 5
# baseline (speedup 1.0000x reference)
"""Trainium2 Bass kernel for nn_ConvM_Layer (episode covariance similarity).

Math reformulation (exact):
  cov      = S_c S_c^T / (hw-1)  with S_c the per-(t,way) centered support (c x 500)
  cov_sim  = q^T cov q = ||S_c^T q||^2 / (hw-1)  >= 0   (PSD quadratic form)
  => LeakyReLU is the identity, and
  score[t,q,w] = sum_p conv_w[p]/(hw-1) * ||S_c^T (q_p - qbar)||^2 + conv_b

Sharding: 8 cores = (t in 0..3) x (wq half in 0..1); wq padded 75 -> 76 = 2*38.
Each core computes its (t, half) shard independently; host gathers.
"""

from contextlib import ExitStack

import numpy as np

import concourse.bass as bass
import concourse.tile as tile
from concourse import bacc, mybir
from concourse.bass_utils import run_bass_kernel_spmd

# Problem shape (hardcoded per contract)
T, WQ, C, H, W = 4, 75, 640, 10, 10
HW = H * W                 # 100
WAY, SHOT = 5, 5
M = SHOT * HW              # 500 support samples per way
WQP = 76                   # padded query count (divisible by 2)
WQH = WQP // 2             # 38 queries per core
NQ = WQH * HW              # 3800 query spatial columns per core
CT = C // 128              # 5 contraction tiles
N_CORES = 8

F32 = mybir.dt.float32
# Matmul compute dtype: float32r streams at bf16 rate (1 cyc/row for N>=256)
# while carrying fp32 data; fall back to F32 (exact, 4x slower) or bfloat16.
DT_MM = mybir.dt.float32r

_CACHE: dict = {}


def _bitcast(ap, dt):
    return ap if dt == F32 else ap.bitcast(dt)


def _kernel_body(ctx: ExitStack, tc: tile.TileContext, q_d, s_d, w_d, o_d):
    nc = tc.nc
    X = mybir.AxisListType.X
    mm_store = DT_MM

    qraw_p = ctx.enter_context(tc.tile_pool(name="qraw", bufs=2))
    qc_p = ctx.enter_context(tc.tile_pool(name="qc", bufs=CT))
    sraw_p = ctx.enter_context(tc.tile_pool(name="sraw", bufs=2))
    sc_p = ctx.enter_context(tc.tile_pool(name="sc", bufs=WAY * CT))
    stat_p = ctx.enter_context(tc.tile_pool(name="stat", bufs=4))
    trash_p = ctx.enter_context(tc.tile_pool(name="trash", bufs=2))
    lcs_p = ctx.enter_context(tc.tile_pool(name="lcs", bufs=1))
    w_p = ctx.enter_context(tc.tile_pool(name="wgt", bufs=1))
    osb_p = ctx.enter_context(tc.tile_pool(name="osb", bufs=1))
    ps_p = ctx.enter_context(tc.tile_pool(name="ps", bufs=4, space="PSUM"))
    ops_p = ctx.enter_context(tc.tile_pool(name="ops", bufs=1, space="PSUM"))

    # conv weights column [HW, 1]
    w_sb = w_p.tile([HW, 1], F32)
    nc.sync.dma_start(w_sb[:], w_d[:])

    # ---- support: load + center per (way, ctile) ----
    s_c = []
    for wy in range(WAY):
        row = []
        for ct in range(CT):
            sraw = sraw_p.tile([128, M], F32)
            src = s_d[wy * SHOT:(wy + 1) * SHOT, ct * 128:(ct + 1) * 128, :]
            nc.sync.dma_start(
                sraw[:].rearrange("c (s h) -> c s h", s=SHOT),
                src.rearrange("s c h -> c s h"),
            )
            smean = stat_p.tile([128, 1], F32, tag="smean")
            nc.vector.reduce_sum(smean[:], sraw[:], axis=X)
            nc.scalar.mul(smean[:], smean[:], 1.0 / M)
            sc = sc_p.tile([128, M], mm_store)
            nc.vector.tensor_scalar_sub(sc[:], sraw[:], smean[:])
            row.append(sc)
        s_c.append(row)

    # ---- query: load + center per ctile ----
    q_c = []
    for ct in range(CT):
        qraw = qraw_p.tile([128, NQ], F32)
        src = q_d[:, ct * 128:(ct + 1) * 128, :]
        nc.sync.dma_start(
            qraw[:].rearrange("c (q h) -> c q h", q=WQH),
            src.rearrange("q c h -> c q h"),
        )
        qmean = stat_p.tile([128, WQH], F32, tag="qmean")
        nc.vector.reduce_sum(
            qmean[:], qraw[:].rearrange("c (q h) -> c q h", q=WQH), axis=X
        )
        nc.scalar.mul(qmean[:], qmean[:], 1.0 / HW)
        qc = qc_p.tile([128, NQ], mm_store)
        for qi in range(WQH):
            sl = slice(qi * HW, (qi + 1) * HW)
            nc.vector.tensor_scalar_sub(
                qc[:, sl], qraw[:, sl], qmean[:, qi:qi + 1]
            )
        q_c.append(qc)

    # ---- main: P = S_c^T Q_q per (way, query); cov_sim col = rowwise ||.||^2 ----
    lcs = lcs_p.tile([HW, WAY * WQH], F32)
    for wy in range(WAY):
        for qi in range(WQH):
            ps = ps_p.tile([HW, M], F32)
            for ct in range(CT):
                nc.tensor.matmul(
                    ps[:],
                    q_c[ct][:, qi * HW:(qi + 1) * HW],
                    s_c[wy][ct][:],
                    start=(ct == 0),
                    stop=(ct == CT - 1),
                )
            trash = trash_p.tile([HW, M], F32)
            col = wy * WQH + qi
            nc.scalar.activation(
                trash[:], ps[:], mybir.ActivationFunctionType.Square,
                accum_out=lcs[:, col:col + 1],
            )

    # ---- score row = conv_w^T @ lcs  -> [1, WAY*WQH] ----
    ops = ops_p.tile([1, WAY * WQH], F32)
    nc.tensor.matmul(ops[:], w_sb[:], lcs[:], start=True, stop=True)
    osb = osb_p.tile([1, WAY * WQH], F32)
    nc.scalar.copy(osb[:], ops[:])
    nc.sync.dma_start(o_d[:], osb[:])


def _build():
    key = "nc"
    if key in _CACHE:
        return _CACHE[key]
    nc = bacc.Bacc(
        "TRN2", target_bir_lowering=False, debug=False, num_devices=N_CORES
    )
    q_d = nc.dram_tensor("q", [WQH, C, HW], F32, kind="ExternalInput").ap()
    s_d = nc.dram_tensor("s", [WAY * SHOT, C, HW], F32, kind="ExternalInput").ap()
    w_d = nc.dram_tensor("w", [HW, 1], F32, kind="ExternalInput").ap()
    o_d = nc.dram_tensor("out", [1, WAY * WQH], F32, kind="ExternalOutput").ap()
    with tile.TileContext(nc) as tc:
        with ExitStack() as ctx:
            _kernel_body(ctx, tc, q_d, s_d, w_d, o_d)
    nc.compile()
    _CACHE[key] = nc
    return nc


def make_in_maps(query_feat, support_feat, conv_w):
    qp = np.zeros((T, WQP, C, HW), dtype=np.float32)
    qp[:, :WQ] = np.asarray(query_feat, dtype=np.float32).reshape(T, WQ, C, HW)
    sp = np.ascontiguousarray(
        np.asarray(support_feat, dtype=np.float32).reshape(T, WAY * SHOT, C, HW)
    )
    w_col = np.ascontiguousarray(
        (np.asarray(conv_w, dtype=np.float32)[0, 0] / (HW - 1)).reshape(HW, 1)
    )
    in_maps = []
    for core in range(N_CORES):
        ti, half = core // 2, core % 2
        in_maps.append({
            "q": np.ascontiguousarray(qp[ti, half * WQH:(half + 1) * WQH]),
            "s": sp[ti],
            "w": w_col,
        })
    return in_maps


LAST_RESULT = None  # set by kernel(); lets a harness read exec_time_ns/profile


def kernel(query_feat, support_feat, conv_w, conv_b):
    global LAST_RESULT
    nc = _build()
    in_maps = make_in_maps(query_feat, support_feat, conv_w)
    res = run_bass_kernel_spmd(nc, in_maps, list(range(N_CORES)))
    LAST_RESULT = res
    score = np.empty((T, WQP, WAY), dtype=np.float32)
    for core in range(N_CORES):
        ti, half = core // 2, core % 2
        row = res.results[core]["out"][0]  # [WAY*WQH]
        score[ti, half * WQH:(half + 1) * WQH, :] = row.reshape(WAY, WQH).T
    out = score[:, :WQ, :] + np.asarray(conv_b, dtype=np.float32)[0]
    return np.ascontiguousarray(out)


# revision 9
# speedup vs baseline: 1.1572x; 1.1572x over previous
"""Trainium2 Bass kernel for nn_ConvM_Layer (episode covariance similarity).

Math reformulation (exact):
  cov      = S_c S_c^T / (hw-1)  with S_c the per-(t,way) centered support (c x 500)
  cov_sim  = q^T cov q = ||S_c^T q||^2 / (hw-1)  >= 0   (PSD quadratic form)
  => LeakyReLU is the identity, and
  score[t,q,w] = sum_p conv_w[p]/(hw-1) * ||S_c^T (q_p - qbar)||^2 + conv_b

Sharding: 8 cores = (t in 0..3) x (wq half in 0..1); wq padded 75 -> 76 = 2*38.
Each core computes its (t, half) shard independently; host gathers.

Inputs are pre-transposed on host to channel-major so every DMA row is a
single contiguous burst (>=3.2KB descriptors instead of 400B).
"""

from contextlib import ExitStack

import numpy as np

import concourse.bass as bass
import concourse.tile as tile
from concourse import bacc, mybir
from concourse.bass_utils import run_bass_kernel_spmd

# Problem shape (hardcoded per contract)
T, WQ, C, H, W = 4, 75, 640, 10, 10
HW = H * W                 # 100
WAY, SHOT = 5, 5
M = SHOT * HW              # 500 support samples per way
WQP = 76                   # padded query count (divisible by 2)
WQH = WQP // 2             # 38 queries per core
NQ = WQH * HW              # 3800 query spatial columns per core
CT = C // 128              # 5 contraction tiles
N_CORES = 8
QCH = 10                   # queries per DMA/compute chunk

F32 = mybir.dt.float32
# Matmul compute dtype: float32r streams at ~1 cyc/row for N>=256 (vs 4 for
# f32) and measured ~1.5e-4 rel err end-to-end on HW.
DT_MM = mybir.dt.float32r

_CACHE: dict = {}


def _chunks():
    out = []
    q0 = 0
    while q0 < WQH:
        out.append((q0, min(QCH, WQH - q0)))
        q0 += QCH
    return out


def _kernel_body(ctx: ExitStack, tc: tile.TileContext, q_d, s_d, w_d, o_d):
    nc = tc.nc
    X = mybir.AxisListType.X

    sraw_p = ctx.enter_context(tc.tile_pool(name="sraw", bufs=3))
    sc_p = ctx.enter_context(tc.tile_pool(name="sc", bufs=WAY * CT))
    qraw_p = ctx.enter_context(tc.tile_pool(name="qraw", bufs=3))
    qc_p = ctx.enter_context(tc.tile_pool(name="qc", bufs=1))
    stat_p = ctx.enter_context(tc.tile_pool(name="stat", bufs=6))
    trash_p = ctx.enter_context(tc.tile_pool(name="trash", bufs=2))
    lcs_p = ctx.enter_context(tc.tile_pool(name="lcs", bufs=1))
    w_p = ctx.enter_context(tc.tile_pool(name="wgt", bufs=1))
    osb_p = ctx.enter_context(tc.tile_pool(name="osb", bufs=1))
    ps_p = ctx.enter_context(tc.tile_pool(name="ps", bufs=6, space="PSUM"))
    ops_p = ctx.enter_context(tc.tile_pool(name="ops", bufs=1, space="PSUM"))

    # conv weights column [HW, 1]
    w_sb = w_p.tile([HW, 1], F32)
    nc.sync.dma_start(w_sb[:], w_d[:])

    # ---- support: load + center per (way, ctile) ----
    s_c = []
    for wy in range(WAY):
        row = []
        for ct in range(CT):
            sraw = sraw_p.tile([128, M], F32)
            nc.sync.dma_start(
                sraw[:], s_d[ct * 128:(ct + 1) * 128, wy * M:(wy + 1) * M]
            )
            smean = stat_p.tile([128, 1], F32, tag="smean")
            nc.vector.reduce_sum(smean[:], sraw[:], axis=X)
            nc.vector.tensor_scalar_mul(smean[:], smean[:], 1.0 / M)
            sc = sc_p.tile([128, M], DT_MM)
            nc.vector.tensor_scalar_sub(sc[:], sraw[:], smean[:])
            row.append(sc)
        s_c.append(row)

    # persistent centered-query tiles, filled chunk by chunk
    q_c = [
        qc_p.tile([128, NQ], DT_MM, name=f"qc{ct}", tag=f"qc{ct}")
        for ct in range(CT)
    ]
    lcs = lcs_p.tile([HW, WAY * WQH], F32)

    for q0, nq in _chunks():
        cols = slice(q0 * HW, (q0 + nq) * HW)
        for ct in range(CT):
            qraw = qraw_p.tile([128, QCH * HW], F32)
            nc.sync.dma_start(qraw[:, :nq * HW], q_d[ct * 128:(ct + 1) * 128, cols])
            qsum = stat_p.tile([128, QCH], F32, tag="qsum")
            nc.vector.reduce_sum(
                qsum[:, :nq],
                qraw[:, :nq * HW].rearrange("c (q h) -> c q h", h=HW),
                axis=X,
            )
            nc.vector.tensor_scalar_mul(qsum[:, :nq], qsum[:, :nq], 1.0 / HW)
            nc.vector.tensor_sub(
                q_c[ct][:, cols].rearrange("c (q h) -> c q h", h=HW),
                qraw[:, :nq * HW].rearrange("c (q h) -> c q h", h=HW),
                qsum[:, :nq].broadcast_to((128, nq, HW)),
            )

        # ---- main: P = S_c^T Q_q per (way, query); cs col = rowwise ||.||^2 ----
        for wy in range(WAY):
            for qi in range(q0, q0 + nq):
                ps = ps_p.tile([HW, M], F32)
                for ct in range(CT):
                    nc.tensor.matmul(
                        ps[:],
                        q_c[ct][:, qi * HW:(qi + 1) * HW],
                        s_c[wy][ct][:],
                        start=(ct == 0),
                        stop=(ct == CT - 1),
                    )
                trash = trash_p.tile([HW, M], F32)
                col = wy * WQH + qi
                nc.scalar.activation(
                    trash[:], ps[:], mybir.ActivationFunctionType.Square,
                    accum_out=lcs[:, col:col + 1],
                )

    # ---- score row = conv_w^T @ lcs  -> [1, WAY*WQH] ----
    ops = ops_p.tile([1, WAY * WQH], F32)
    nc.tensor.matmul(ops[:], w_sb[:], lcs[:], start=True, stop=True)
    osb = osb_p.tile([1, WAY * WQH], F32)
    nc.scalar.copy(osb[:], ops[:])
    nc.sync.dma_start(o_d[:], osb[:])


def _build():
    key = "nc"
    if key in _CACHE:
        return _CACHE[key]
    nc = bacc.Bacc(
        "TRN2", target_bir_lowering=False, debug=False, num_devices=N_CORES
    )
    q_d = nc.dram_tensor("q", [C, NQ], F32, kind="ExternalInput").ap()
    s_d = nc.dram_tensor("s", [C, WAY * M], F32, kind="ExternalInput").ap()
    w_d = nc.dram_tensor("w", [HW, 1], F32, kind="ExternalInput").ap()
    o_d = nc.dram_tensor("out", [1, WAY * WQH], F32, kind="ExternalOutput").ap()
    with tile.TileContext(nc) as tc:
        with ExitStack() as ctx:
            _kernel_body(ctx, tc, q_d, s_d, w_d, o_d)
    nc.compile()
    _CACHE[key] = nc
    return nc


def make_in_maps(query_feat, support_feat, conv_w):
    q = np.asarray(query_feat, dtype=np.float32).reshape(T, WQ, C, HW)
    s = np.asarray(support_feat, dtype=np.float32).reshape(T, WAY * SHOT, C, HW)
    w_col = np.ascontiguousarray(
        (np.asarray(conv_w, dtype=np.float32)[0, 0] / (HW - 1)).reshape(HW, 1)
    )
    # channel-major transposes so every DMA partition-row is contiguous
    qt = np.zeros((T, C, WQP * HW), dtype=np.float32)
    qt[:, :, :WQ * HW] = q.transpose(0, 2, 1, 3).reshape(T, C, WQ * HW)
    st = np.ascontiguousarray(s.transpose(0, 2, 1, 3).reshape(T, C, WAY * M))
    in_maps = []
    for core in range(N_CORES):
        ti, half = core // 2, core % 2
        in_maps.append({
            "q": np.ascontiguousarray(qt[ti, :, half * NQ:(half + 1) * NQ]),
            "s": st[ti],
            "w": w_col,
        })
    return in_maps


LAST_RESULT = None  # set by kernel(); lets a harness read exec_time_ns/profile


def kernel(query_feat, support_feat, conv_w, conv_b):
    global LAST_RESULT
    nc = _build()
    in_maps = make_in_maps(query_feat, support_feat, conv_w)
    res = run_bass_kernel_spmd(nc, in_maps, list(range(N_CORES)))
    LAST_RESULT = res
    score = np.empty((T, WQP, WAY), dtype=np.float32)
    for core in range(N_CORES):
        ti, half = core // 2, core % 2
        row = res.results[core]["out"][0]  # [WAY*WQH]
        score[ti, half * WQH:(half + 1) * WQH, :] = row.reshape(WAY, WQH).T
    out = score[:, :WQ, :] + np.asarray(conv_b, dtype=np.float32)[0]
    return np.ascontiguousarray(out)
